# revision 1
# baseline (speedup 1.0000x reference)
"""Trainium2 Bass kernel for ChamferLoss (B=8, C=3, N=4096), 8 NeuronCores.

Strategy: data-parallel over batch. Core b computes batch b fully:
  D[n,m] = ||x_n||^2 + ||y_m||^2 - 2 x_n.y_m   (x = ori, y = adv points)
  d1 = mean_n relu(min_m D),  d2 = mean_m relu(min_n D)
Host combines: mean_b max(d1_b, d2_b).

The -2*x.y matmul has contraction K=3; fp32 matmul is 4x slower on PE, so
each fp32 value v is split v = vh + vl (bf16 pair) and the product uses the
3-term expansion  x.y ~= xh.yh + xh.yl + xl.yh  (error ~2^-16 relative).
The squared norms are folded into the same matmul via constant-one rows, so
PSUM holds complete distance values and one reduce-min per row-block gives
the row minima. Column minima come from a second pass with roles swapped.
"""

import sys

sys.path.insert(0, "/opt/trn_rl_repo")

import numpy as np

import concourse.bass as bass  # noqa: F401  (registers engine types)
import concourse.tile as tile
from concourse import bacc, bass_utils, mybir

B, C, N = 8, 3, 4096
NCORES = 8
NO = 32  # n_outer blocks of 128
NI = 128  # n_inner
F32 = mybir.dt.float32
BF16 = mybir.dt.bfloat16
K = 13  # contraction rows: 9 coord product terms + 2 sq rows + 2 one rows

_CACHE = {}


def _prep_pointset(nc, tc, sb, rr, v_dram):
    """Load [3, 4096] fp32 points; return dict of packed SBUF tiles.

    Layouts: vh/vl/m2h/m2l are [96, 128] bf16 (partition = 32*c + n_outer,
    free = n_inner). v2h/v2l are [32, 128] bf16 (partition = n_outer).
    """
    vp = sb.tile([96, 128], F32)
    nc.sync.dma_start(vp[:], v_dram.rearrange("c (no ni) -> (c no) ni", ni=NI))

    vh = sb.tile([96, 128], BF16)
    nc.vector.tensor_copy(vh[:], vp[:])
    vl = sb.tile([96, 128], BF16)
    nc.vector.tensor_sub(vl[:], vp[:], vh[:])
    m2h = sb.tile([96, 128], BF16)
    nc.vector.tensor_scalar_mul(m2h[:], vh[:], -2.0)
    m2l = sb.tile([96, 128], BF16)
    nc.vector.tensor_scalar_mul(m2l[:], vl[:], -2.0)

    vsq = sb.tile([96, 128], F32)
    nc.vector.tensor_mul(vsq[:], vp[:], vp[:])
    # gather the three c-blocks side by side on partitions 0..31
    vsqr = sb.tile([32, 384], F32)
    for c in range(3):
        rr.dma(vsqr[:, 128 * c : 128 * (c + 1)], vsq[32 * c : 32 * (c + 1), :])
    v2 = sb.tile([32, 128], F32)
    nc.vector.tensor_add(v2[:], vsqr[:, 0:128], vsqr[:, 128:256])
    nc.vector.tensor_add(v2[:], v2[:], vsqr[:, 256:384])
    v2h = sb.tile([32, 128], BF16)
    nc.vector.tensor_copy(v2h[:], v2[:])
    v2l = sb.tile([32, 128], BF16)
    nc.vector.tensor_sub(v2l[:], v2[:], v2h[:])
    return dict(vh=vh, vl=vl, m2h=m2h, m2l=m2l, v2h=v2h, v2l=v2l)


class _DmaRR:
    """Round-robin DMA issue across several engines so prep/assembly
    transfers land on different hardware queues and run in parallel."""

    def __init__(self, nc):
        self.engines = [nc.sync, nc.scalar, nc.gpsimd]
        self.i = 0

    def dma(self, out, in_):
        e = self.engines[self.i % len(self.engines)]
        self.i += 1
        e.dma_start(out, in_)


def _assemble_lhs(nc, rr, sb, p, ones64, name):
    """lhsT image [13, 4096] bf16, row order chosen so each source tile
    lands with ONE contiguous 3-row DMA:
      rows 0-2 = m2h_c, rows 3-5 = m2h_c (again), rows 6-8 = m2l_c,
      rows 9,10 = ones, rows 11,12 = v2h, v2l."""
    m = sb.tile([128, N], BF16, name=name)
    rr.dma(m[0:3, :], p["m2h"][:])
    rr.dma(m[3:6, :], p["m2h"][:])
    rr.dma(m[6:9, :], p["m2l"][:])
    rr.dma(m[9:11, :], ones64[:])
    rr.dma(m[11:12, :], p["v2h"][:])
    rr.dma(m[12:13, :], p["v2l"][:])
    return m


def _assemble_rhs(nc, rr, sb, p, ones64, name):
    """rhs image [13, 4096] bf16 pairing the lhs row order:
      rows 0-2 = vh_c, rows 3-5 = vl_c, rows 6-8 = vh_c,
      rows 9,10 = v2h, v2l, rows 11,12 = ones."""
    m = sb.tile([128, N], BF16, name=name)
    rr.dma(m[0:3, :], p["vh"][:])
    rr.dma(m[3:6, :], p["vl"][:])
    rr.dma(m[6:9, :], p["vh"][:])
    rr.dma(m[9:10, :], p["v2h"][:])
    rr.dma(m[10:11, :], p["v2l"][:])
    rr.dma(m[11:13, :], ones64[:])
    return m


def _build():
    nc = bacc.Bacc("TRN2", target_bir_lowering=False, debug=False)
    x_d = nc.dram_tensor("x", [C, N], F32, kind="ExternalInput").ap()
    y_d = nc.dram_tensor("y", [C, N], F32, kind="ExternalInput").ap()
    out_d = nc.dram_tensor("o", [128, 2], F32, kind="ExternalOutput").ap()

    with tile.TileContext(nc) as tc:
        with (
            tc.tile_pool(name="prep", bufs=1) as prep,
            tc.tile_pool(name="mats", bufs=1) as mats,
            tc.tile_pool(name="parts", bufs=1) as parts,
            tc.tile_pool(name="psum", bufs=2, space="PSUM") as psum,
        ):
            rr = _DmaRR(nc)
            px = _prep_pointset(nc, tc, prep, rr, x_d)
            py = _prep_pointset(nc, tc, prep, rr, y_d)
            ones64 = prep.tile([64, 128], BF16)
            nc.gpsimd.memset(ones64[:], 1.0)

            # Replicas at partition offsets 32/64/96 let four PE row-groups
            # run concurrent matmuls (tile_position), each filling a
            # different PSUM bank of the same unit: ~4x PE throughput for
            # these tiny-K matmuls. LX/RY are assembled (and replicated)
            # first so direction-0 matmuls can start while RX/LY DMAs are
            # still in flight behind them in the queues.
            def replicate(m):
                for t in range(1, 4):
                    rr.dma(m[32 * t : 32 * t + K, :], m[0:K, :])

            LX = _assemble_lhs(nc, rr, mats, px, ones64, "LX")
            RY = _assemble_rhs(nc, rr, mats, py, ones64, "RY")
            replicate(LX)
            replicate(RY)
            LY = _assemble_lhs(nc, rr, mats, py, ones64, "LY")
            RX = _assemble_rhs(nc, rr, mats, px, ones64, "RX")
            replicate(LY)
            replicate(RX)

            partials = []
            for d in range(2):
                pt = parts.tile([128, 64], F32, name=f"part{d}")
                nc.vector.memset(pt[:], 3.0e38)
                partials.append(pt)

            # Each (r, direction) covers a [128, 4096] slab of D, computed as
            # two [128, 2048] PSUM units (h = 0, 1). The four 512-col banks
            # of a unit are filled by four concurrent PE row-group matmuls
            # (tile_position) using the replicated matrix rows.
            def fill_unit(L, R, r, h):
                p = psum.tile([128, 2048], F32, name="pp")
                for j in range(4):
                    nc.tensor.matmul(
                        p[:, 512 * j : 512 * (j + 1)],
                        L[32 * j : 32 * j + K, 128 * r : 128 * (r + 1)],
                        R[32 * j : 32 * j + K,
                          2048 * h + 512 * j : 2048 * h + 512 * (j + 1)],
                        start=True,
                        stop=True,
                        tile_position=(32 * j, 0),
                    )
                return p

            def fold_tree(pm, col, work):
                # pm: [128, 2048] bf16 holding min(h0, h1) -> rowmin into col
                t1 = work.tile([128, 1024], BF16, name="t1")
                nc.vector.tensor_tensor(
                    out=t1[:], in0=pm[:, 0:1024], in1=pm[:, 1024:2048],
                    op=mybir.AluOpType.min)
                t2 = work.tile([128, 512], BF16, name="t2")
                nc.vector.tensor_tensor(
                    out=t2[:], in0=t1[:, 0:512], in1=t1[:, 512:1024],
                    op=mybir.AluOpType.min)
                t3 = work.tile([128, 256], BF16, name="t3")
                nc.vector.tensor_tensor(
                    out=t3[:], in0=t2[:, 0:256], in1=t2[:, 256:512],
                    op=mybir.AluOpType.min)
                nc.vector.tensor_reduce(
                    col, t3[:], axis=mybir.AxisListType.X, op=mybir.AluOpType.min
                )

            # Drain routing per (r, d) pair of units, two flavors mixed to
            # balance the only two engines that can read PSUM:
            #  P-pair: ACT copies h0 to bf16; one 1x DVE tensor_tensor min
            #          drains h1 AND folds it with h0's copy in a single op.
            #  D-pair: ACT copies both halves; DVE folds at 2x bf16 rate.
            def emit_pair(kind, L, R, r, pt, work):
                col = pt[:, 2 * r : 2 * r + 1]
                p0 = fill_unit(L, R, r, 0)
                c0 = work.tile([128, 2048], BF16, name="c0")
                nc.scalar.copy(c0[:], p0[:])
                p1 = fill_unit(L, R, r, 1)
                pm = work.tile([128, 2048], BF16, name="pm")
                if kind == "P":
                    nc.vector.tensor_tensor(
                        out=pm[:], in0=p1[:], in1=c0[:], op=mybir.AluOpType.min
                    )
                else:
                    c1 = work.tile([128, 2048], BF16, name="c1")
                    nc.scalar.copy(c1[:], p1[:])
                    nc.vector.tensor_tensor(
                        out=pm[:], in0=c0[:], in1=c1[:], op=mybir.AluOpType.min
                    )
                fold_tree(pm, col, work)

            pattern = "PDPDPDPDD"
            with tc.tile_pool(name="work", bufs=6) as work:
                idx = 0
                for d, (L, R) in enumerate(((LX, RY), (LY, RX))):
                    for r in range(NO):
                        kind = pattern[idx % len(pattern)]
                        emit_pair(kind, L, R, r, partials[d], work)
                        idx += 1

            osb = parts.tile([128, 2], F32)
            for d in range(2):
                rm = parts.tile([128, 32], F32, name=f"rm{d}")
                nc.vector.tensor_reduce(
                    rm[:],
                    partials[d][:].rearrange("p (no h) -> p no h", h=2),
                    axis=mybir.AxisListType.X,
                    op=mybir.AluOpType.min,
                )
                nc.vector.tensor_scalar_max(rm[:], rm[:], 0.0)
                nc.vector.reduce_sum(
                    osb[:, d : d + 1], rm[:], axis=mybir.AxisListType.X
                )
            nc.sync.dma_start(out_d[:], osb[:])

    nc.compile()
    return nc


def kernel(ori_pcs: np.ndarray, adv_pcs: np.ndarray) -> np.ndarray:
    if "nc" not in _CACHE:
        _CACHE["nc"] = _build()
    nc = _CACHE["nc"]

    ori = np.ascontiguousarray(np.asarray(ori_pcs, dtype=np.float32))
    adv = np.ascontiguousarray(np.asarray(adv_pcs, dtype=np.float32))
    in_maps = [{"x": ori[b], "y": adv[b]} for b in range(B)]
    res = bass_utils.run_bass_kernel_spmd(nc, in_maps, core_ids=list(range(NCORES)))

    vals = []
    for b in range(B):
        o = res.results[b]["o"].astype(np.float64)
        d1 = o[:, 0].sum() / N
        d2 = o[:, 1].sum() / N
        vals.append(max(d1, d2))
    return np.array(np.mean(vals), dtype=np.float32)



# revision 8
# speedup vs baseline: 1.1447x; 1.1447x over previous
"""Trainium2 Bass kernel for ChamferLoss (B=8, C=3, N=4096), 8 NeuronCores.

Strategy: data-parallel over batch; core b computes batch b fully.
  D[n,m] = ||x_n||^2 + ||y_m||^2 - 2 x_n.y_m   (x = ori, y = adv points)
  d1 = mean_n relu(min_m D),  d2 = mean_m relu(min_n D)
Host combines: mean_b max(d1_b, d2_b).

Unlike the two-pass variant (which re-computed D transposed for the
column direction), D is computed ONCE per slab of 128 rows:
  - ACT drains each PSUM unit to bf16 SBUF with a fused relu (the only
    engines that read PSUM are ACT and DVE, at 1 elem/cycle/partition,
    so halving PSUM traffic is the main win),
  - DVE computes the slab row-min with one fused tensor_tensor_reduce
    and folds the slab into a running column-min accumulator with one
    tensor_tensor min,
  - the column direction finishes with 32 PE transposes of the
    accumulator + per-tile min reduces.

The -2*x.y matmul has contraction K=3; fp32 matmul is 4x slower on PE, so
each fp32 value v is split v = vh + vl (bf16 pair) and the product uses the
3-term expansion  x.y ~= xh.yh + xh.yl + xl.yh  (error ~2^-16 relative).
The squared norms are folded into the same matmul via constant-one rows, so
PSUM holds complete distance values.
"""

import sys

sys.path.insert(0, "/opt/trn_rl_repo")

import numpy as np

import concourse.bass as bass  # noqa: F401  (registers engine types)
import concourse.tile as tile
from concourse import bacc, bass_utils, masks, mybir

B, C, N = 8, 3, 4096
NCORES = 8
NO = 32  # n_outer blocks of 128 rows
NI = 128  # n_inner
F32 = mybir.dt.float32
BF16 = mybir.dt.bfloat16
K = 13  # contraction rows: 9 coord product terms + 2 sq rows + 2 one rows
BIG = 3.0e38
import os
DEBUG_SKIP_TAIL = bool(int(os.environ.get("DBG_SKIP_TAIL", "0")))
DEBUG_NO_TTR = bool(int(os.environ.get("DBG_NO_TTR", "0")))
DEBUG_COPY_DRAIN = bool(int(os.environ.get("DBG_COPY_DRAIN", "0")))

_CACHE = {}


def _prep_pointset(nc, tc, sb, rr, v_dram):
    """Load [3, 4096] fp32 points; return dict of packed SBUF tiles.

    Layouts: vh/vl/m2h/m2l are [96, 128] bf16 (partition = 32*c + n_outer,
    free = n_inner). v2h/v2l are [32, 128] bf16 (partition = n_outer).
    """
    vp = sb.tile([96, 128], F32)
    nc.sync.dma_start(vp[:], v_dram.rearrange("c (no ni) -> (c no) ni", ni=NI))

    vh = sb.tile([96, 128], BF16)
    nc.vector.tensor_copy(vh[:], vp[:])
    vl = sb.tile([96, 128], BF16)
    nc.vector.tensor_sub(vl[:], vp[:], vh[:])
    m2h = sb.tile([96, 128], BF16)
    nc.vector.tensor_scalar_mul(m2h[:], vh[:], -2.0)
    m2l = sb.tile([96, 128], BF16)
    nc.vector.tensor_scalar_mul(m2l[:], vl[:], -2.0)

    vsq = sb.tile([96, 128], F32)
    nc.vector.tensor_mul(vsq[:], vp[:], vp[:])
    # gather the three c-blocks side by side on partitions 0..31
    vsqr = sb.tile([32, 384], F32)
    for c in range(3):
        rr.dma(vsqr[:, 128 * c : 128 * (c + 1)], vsq[32 * c : 32 * (c + 1), :])
    v2 = sb.tile([32, 128], F32)
    nc.vector.tensor_add(v2[:], vsqr[:, 0:128], vsqr[:, 128:256])
    nc.vector.tensor_add(v2[:], v2[:], vsqr[:, 256:384])
    v2h = sb.tile([32, 128], BF16)
    nc.vector.tensor_copy(v2h[:], v2[:])
    v2l = sb.tile([32, 128], BF16)
    nc.vector.tensor_sub(v2l[:], v2[:], v2h[:])
    return dict(vh=vh, vl=vl, m2h=m2h, m2l=m2l, v2h=v2h, v2l=v2l)


class _DmaRR:
    """Round-robin DMA issue across several engines so prep/assembly
    transfers land on different hardware queues and run in parallel."""

    def __init__(self, nc):
        self.engines = [nc.sync, nc.scalar, nc.gpsimd]
        self.i = 0

    def dma(self, out, in_):
        e = self.engines[self.i % len(self.engines)]
        self.i += 1
        e.dma_start(out, in_)


def _assemble_lhs(nc, rr, sb, p, ones64, name):
    """lhsT image [13, 4096] bf16, row order chosen so each source tile
    lands with ONE contiguous 3-row DMA:
      rows 0-2 = m2h_c, rows 3-5 = m2h_c (again), rows 6-8 = m2l_c,
      rows 9,10 = ones, rows 11,12 = v2h, v2l."""
    m = sb.tile([128, N], BF16, name=name)
    rr.dma(m[0:3, :], p["m2h"][:])
    rr.dma(m[3:6, :], p["m2h"][:])
    rr.dma(m[6:9, :], p["m2l"][:])
    rr.dma(m[9:11, :], ones64[:])
    rr.dma(m[11:12, :], p["v2h"][:])
    rr.dma(m[12:13, :], p["v2l"][:])
    return m


def _assemble_rhs(nc, rr, sb, p, ones64, name):
    """rhs image [13, 4096] bf16 pairing the lhs row order:
      rows 0-2 = vh_c, rows 3-5 = vl_c, rows 6-8 = vh_c,
      rows 9,10 = v2h, v2l, rows 11,12 = ones."""
    m = sb.tile([128, N], BF16, name=name)
    rr.dma(m[0:3, :], p["vh"][:])
    rr.dma(m[3:6, :], p["vl"][:])
    rr.dma(m[6:9, :], p["vh"][:])
    rr.dma(m[9:10, :], p["v2h"][:])
    rr.dma(m[10:11, :], p["v2l"][:])
    rr.dma(m[11:13, :], ones64[:])
    return m


def _build():
    nc = bacc.Bacc("TRN2", target_bir_lowering=False, debug=False)
    x_d = nc.dram_tensor("x", [C, N], F32, kind="ExternalInput").ap()
    y_d = nc.dram_tensor("y", [C, N], F32, kind="ExternalInput").ap()
    out_d = nc.dram_tensor("o", [128, 2], F32, kind="ExternalOutput").ap()

    with tile.TileContext(nc) as tc:
        with (
            tc.tile_pool(name="prep", bufs=1) as prep,
            tc.tile_pool(name="mats", bufs=1) as mats,
            tc.tile_pool(name="parts", bufs=1) as parts,
        ):
            rr = _DmaRR(nc)
            px = _prep_pointset(nc, tc, prep, rr, x_d)
            py = _prep_pointset(nc, tc, prep, rr, y_d)
            ones64 = prep.tile([64, 128], BF16)
            nc.gpsimd.memset(ones64[:], 1.0)

            # Replicas at partition offsets 32/64/96 let four PE row-groups
            # run concurrent matmuls (tile_position), each filling a
            # different PSUM bank of the same unit: ~4x PE throughput for
            # these tiny-K matmuls.
            def replicate(m):
                for t in range(1, 4):
                    rr.dma(m[32 * t : 32 * t + K, :], m[0:K, :])

            LX = _assemble_lhs(nc, rr, mats, px, ones64, "LX")
            RY = _assemble_rhs(nc, rr, mats, py, ones64, "RY")
            replicate(LX)
            replicate(RY)

            identity = parts.tile([128, 128], BF16)
            masks.make_identity(nc, identity[:])

            acc = parts.tile([128, N], BF16)
            nc.vector.memset(acc[:], BIG)
            rowpart = parts.tile([128, NO], F32)
            colpart = parts.tile([128, NO], F32)

            # Each slab r covers rows [128r, 128r+128) of D as two
            # [128, 2048] PSUM units, each filled by four concurrent PE
            # row-group matmuls (tile_position) using the replicated rows.
            def fill_unit(r, h):
                p = psum.tile([128, 2048], F32, name="pp")
                for j in range(4):
                    nc.tensor.matmul(
                        p[:, 512 * j : 512 * (j + 1)],
                        LX[32 * j : 32 * j + K, 128 * r : 128 * (r + 1)],
                        RY[32 * j : 32 * j + K,
                           2048 * h + 512 * j : 2048 * h + 512 * (j + 1)],
                        start=True,
                        stop=True,
                        tile_position=(32 * j, 0),
                    )
                return p

            with (
                tc.tile_pool(name="psum", bufs=2, space="PSUM") as psum,
                tc.tile_pool(name="drain", bufs=3) as drain,
                tc.tile_pool(name="scr", bufs=2) as scr,
            ):
                for r in range(NO):
                    c = drain.tile([128, N], BF16, name="c")
                    for h in range(2):
                        p = fill_unit(r, h)
                        if DEBUG_COPY_DRAIN:
                            nc.scalar.copy(c[:, 2048 * h : 2048 * (h + 1)], p[:])
                        else:
                            nc.scalar.activation(
                                c[:, 2048 * h : 2048 * (h + 1)],
                                p[:],
                                mybir.ActivationFunctionType.Relu,
                            )
                    scratch = scr.tile([128, 2048], BF16, name="scratch")
                    if DEBUG_NO_TTR:
                        nc.vector.tensor_tensor(
                            out=scratch[:], in0=c[:, 0:2048], in1=c[:, 2048:4096],
                            op=mybir.AluOpType.min)
                        t1 = scr.tile([128, 1024], BF16, name="t1")
                        nc.vector.tensor_tensor(
                            out=t1[:], in0=scratch[:, 0:1024], in1=scratch[:, 1024:2048],
                            op=mybir.AluOpType.min)
                        nc.vector.tensor_tensor(
                            out=t1[:, 0:512], in0=t1[:, 0:512], in1=t1[:, 512:1024],
                            op=mybir.AluOpType.min)
                        nc.vector.tensor_reduce(
                            rowpart[:, r : r + 1], t1[:, 0:512],
                            axis=mybir.AxisListType.X, op=mybir.AluOpType.min)
                    else:
                        nc.vector.tensor_tensor_reduce(
                            scratch[:],
                            c[:, 0:2048],
                            c[:, 2048:4096],
                            scale=1.0,
                            scalar=BIG,
                            op0=mybir.AluOpType.min,
                            op1=mybir.AluOpType.min,
                            accum_out=rowpart[:, r : r + 1],
                        )
                    nc.vector.tensor_tensor(
                        out=acc[:], in0=acc[:], in1=c[:], op=mybir.AluOpType.min
                    )

            # Tail: column minima. acc[p, m] = min over slabs; transpose
            # 128-column chunks (PE keeps bf16 into PSUM) and min-reduce
            # each to get colmin per column block.
            if DEBUG_SKIP_TAIL:
                nc.vector.memset(colpart[:], 0.0)
            else:
                with tc.tile_pool(name="tpsum", bufs=4, space="PSUM") as tpsum:
                    for k in range(NO):
                        tp = tpsum.tile([128, 128], BF16, name="tp")
                        nc.tensor.transpose(
                            tp[:], acc[:, 128 * k : 128 * (k + 1)], identity[:]
                        )
                        nc.vector.tensor_reduce(
                            colpart[:, k : k + 1],
                            tp[:],
                            axis=mybir.AxisListType.X,
                            op=mybir.AluOpType.min,
                        )

            osb = parts.tile([128, 2], F32)
            nc.vector.reduce_sum(osb[:, 0:1], rowpart[:], axis=mybir.AxisListType.X)
            nc.vector.reduce_sum(osb[:, 1:2], colpart[:], axis=mybir.AxisListType.X)
            nc.sync.dma_start(out_d[:], osb[:])

    nc.compile()
    return nc


def kernel(ori_pcs: np.ndarray, adv_pcs: np.ndarray) -> np.ndarray:
    if "nc" not in _CACHE:
        _CACHE["nc"] = _build()
    nc = _CACHE["nc"]

    ori = np.ascontiguousarray(np.asarray(ori_pcs, dtype=np.float32))
    adv = np.ascontiguousarray(np.asarray(adv_pcs, dtype=np.float32))
    in_maps = [{"x": ori[b], "y": adv[b]} for b in range(B)]
    res = bass_utils.run_bass_kernel_spmd(nc, in_maps, core_ids=list(range(NCORES)))

    vals = []
    for b in range(B):
        o = res.results[b]["o"].astype(np.float64)
        d1 = o[:, 0].sum() / N
        d2 = o[:, 1].sum() / N
        vals.append(max(d1, d2))
    return np.array(np.mean(vals), dtype=np.float32)


# revision 9
# speedup vs baseline: 1.2269x; 1.0718x over previous
"""Trainium2 Bass kernel for ChamferLoss (B=8, C=3, N=4096), 8 NeuronCores.

Strategy: data-parallel over batch; core b computes batch b fully.
  D[n,m] = ||x_n||^2 + ||y_m||^2 - 2 x_n.y_m   (x = ori, y = adv points)
  d1 = mean_n relu(min_m D),  d2 = mean_m relu(min_n D)
Host combines: mean_b max(d1_b, d2_b).

One-pass design: D is computed once per slab of 128 rows (two [128, 2048]
fp32 PSUM units). The Scalar engine drains each unit to bf16 SBUF (ACT and
DVE are the only engines that can read PSUM, at 1 elem/cycle/partition, so
PSUM traffic is paid exactly once per element). The Vector engine then
  - computes the slab row-min with ONE custom fused DVE op
    (out = min(in0, in1), accum_out = min-reduce of out) — the stock
    TENSOR_TENSOR_REDUCE opcode's firmware table only implements
    mult/add, so a custom table op is registered instead, and
  - folds the slab into a running column-min accumulator with one
    tensor_tensor min.
The column direction finishes with 32 PE transposes of the accumulator
(bf16 PSUM) + per-tile min reduces. relu is applied to the [128, 32]
partials at the end (relu commutes with min).

The -2*x.y matmul has contraction K=3; fp32 matmul is 4x slower on PE, so
each fp32 value v is split v = vh + vl (bf16 pair) and the product uses the
3-term expansion  x.y ~= xh.yh + xh.yl + xl.yh  (error ~2^-16 relative).
The squared norms are folded into the same matmul via constant-one rows, so
PSUM holds complete distance values.
"""

import os
import sys

sys.path.insert(0, "/opt/trn_rl_repo")

import numpy as np

import concourse.bass as bass  # noqa: F401  (registers engine types)
import concourse.tile as tile
from concourse import bacc, bass_utils, masks, mybir

B, C, N = 8, 3, 4096
NCORES = 8
NO = 32  # n_outer blocks of 128 rows
NI = 128  # n_inner
F32 = mybir.dt.float32
BF16 = mybir.dt.bfloat16
K = 13  # contraction rows: 9 coord product terms + 2 sq rows + 2 one rows
BIG = 3.0e38

DEBUG_NO_CUSTOM = bool(int(os.environ.get("DBG_NO_CUSTOM", "0")))

_CACHE = {}


def _register_minmin_op():
    """Register the fused (min, min-reduce) custom DVE op at runtime.

    out = min(in0, in1); accum_out = min(s0, min over free dim of out).
    Uses the documented custom-DVE extension point (dve_ops.OPS +
    per-NEFF table gen); the sha is self-pinned since this op is defined
    here rather than in the repo's dve_ops registry.
    """
    if "minmin" in _CACHE:
        return _CACHE["minmin"]
    from concourse import dve_ops as dops
    from concourse.dve_spec import Spec, Src0, Src1, C0, minn, lower
    from concourse.dve_uop import DveOpSpec

    name = "CHAMFER_MINMIN_RED"

    def _ref(in0, in1, c0, c1, c2):
        o = np.minimum(in0, in1).astype(np.float32)
        a = np.minimum(
            c0, o.reshape(o.shape[0], -1).min(axis=-1, keepdims=True)
        )
        return o, a

    spec = Spec(body=minn(Src0, Src1), accum=minn, accum_init=C0, reference=_ref)
    row = dops._CUSTOM_DVE_ROW_BASE + len(dops.OPS)
    shas = {}
    for ver in ("v3", "v4"):
        try:
            s = DveOpSpec(name=name, opcode=row, uops=lower(spec, ver=ver), rd1_en=True)
            shas[ver] = s.sha(ver)
        except Exception:
            pass
    op = dops.DveOp(name, spec, subdim=False, uops_sha=shas)
    dops.OPS.append(op)
    dops.CUSTOM_DVE_SPECS[name] = spec
    dops._SUB_OPCODE_FOR_NAME[name] = row
    _CACHE["minmin"] = op
    return op


def _prep_pointset(nc, tc, sb, rr, v_dram):
    """Load [3, 4096] fp32 points; return dict of packed SBUF tiles.

    Layouts: vh/vl/m2h/m2l are [96, 128] bf16 (partition = 32*c + n_outer,
    free = n_inner). v2h/v2l are [32, 128] bf16 (partition = n_outer).
    """
    vp = sb.tile([96, 128], F32)
    nc.sync.dma_start(vp[:], v_dram.rearrange("c (no ni) -> (c no) ni", ni=NI))

    vh = sb.tile([96, 128], BF16)
    nc.vector.tensor_copy(vh[:], vp[:])
    vl = sb.tile([96, 128], BF16)
    nc.vector.tensor_sub(vl[:], vp[:], vh[:])
    m2h = sb.tile([96, 128], BF16)
    nc.vector.tensor_scalar_mul(m2h[:], vh[:], -2.0)
    m2l = sb.tile([96, 128], BF16)
    nc.vector.tensor_scalar_mul(m2l[:], vl[:], -2.0)

    vsq = sb.tile([96, 128], F32)
    nc.vector.tensor_mul(vsq[:], vp[:], vp[:])
    # gather the three c-blocks side by side on partitions 0..31
    vsqr = sb.tile([32, 384], F32)
    for c in range(3):
        rr.dma(vsqr[:, 128 * c : 128 * (c + 1)], vsq[32 * c : 32 * (c + 1), :])
    v2 = sb.tile([32, 128], F32)
    nc.vector.tensor_add(v2[:], vsqr[:, 0:128], vsqr[:, 128:256])
    nc.vector.tensor_add(v2[:], v2[:], vsqr[:, 256:384])
    v2h = sb.tile([32, 128], BF16)
    nc.vector.tensor_copy(v2h[:], v2[:])
    v2l = sb.tile([32, 128], BF16)
    nc.vector.tensor_sub(v2l[:], v2[:], v2h[:])
    return dict(vh=vh, vl=vl, m2h=m2h, m2l=m2l, v2h=v2h, v2l=v2l)


class _DmaRR:
    """Round-robin DMA issue across several engines so prep/assembly
    transfers land on different hardware queues and run in parallel."""

    def __init__(self, nc):
        self.engines = [nc.sync, nc.scalar, nc.gpsimd]
        self.i = 0

    def dma(self, out, in_):
        e = self.engines[self.i % len(self.engines)]
        self.i += 1
        e.dma_start(out, in_)


def _assemble_lhs(nc, rr, sb, p, ones64, name):
    """lhsT image [13, 4096] bf16, row order chosen so each source tile
    lands with ONE contiguous 3-row DMA:
      rows 0-2 = m2h_c, rows 3-5 = m2h_c (again), rows 6-8 = m2l_c,
      rows 9,10 = ones, rows 11,12 = v2h, v2l."""
    m = sb.tile([128, N], BF16, name=name)
    rr.dma(m[0:3, :], p["m2h"][:])
    rr.dma(m[3:6, :], p["m2h"][:])
    rr.dma(m[6:9, :], p["m2l"][:])
    rr.dma(m[9:11, :], ones64[:])
    rr.dma(m[11:12, :], p["v2h"][:])
    rr.dma(m[12:13, :], p["v2l"][:])
    return m


def _assemble_rhs(nc, rr, sb, p, ones64, name):
    """rhs image [13, 4096] bf16 pairing the lhs row order:
      rows 0-2 = vh_c, rows 3-5 = vl_c, rows 6-8 = vh_c,
      rows 9,10 = v2h, v2l, rows 11,12 = ones."""
    m = sb.tile([128, N], BF16, name=name)
    rr.dma(m[0:3, :], p["vh"][:])
    rr.dma(m[3:6, :], p["vl"][:])
    rr.dma(m[6:9, :], p["vh"][:])
    rr.dma(m[9:10, :], p["v2h"][:])
    rr.dma(m[10:11, :], p["v2l"][:])
    rr.dma(m[11:13, :], ones64[:])
    return m


def _build():
    minmin = _register_minmin_op()
    nc = bacc.Bacc("TRN2", target_bir_lowering=False, debug=False)
    x_d = nc.dram_tensor("x", [C, N], F32, kind="ExternalInput").ap()
    y_d = nc.dram_tensor("y", [C, N], F32, kind="ExternalInput").ap()
    out_d = nc.dram_tensor("o", [128, 2], F32, kind="ExternalOutput").ap()

    with tile.TileContext(nc) as tc:
        with (
            tc.tile_pool(name="prep", bufs=1) as prep,
            tc.tile_pool(name="mats", bufs=1) as mats,
            tc.tile_pool(name="parts", bufs=1) as parts,
        ):
            rr = _DmaRR(nc)
            px = _prep_pointset(nc, tc, prep, rr, x_d)
            py = _prep_pointset(nc, tc, prep, rr, y_d)
            ones64 = prep.tile([64, 128], BF16)
            nc.gpsimd.memset(ones64[:], 1.0)

            # Replicas at partition offsets 32/64/96 let four PE row-groups
            # run concurrent matmuls (tile_position), each filling a
            # different PSUM bank of the same unit: ~4x PE throughput for
            # these tiny-K matmuls.
            def replicate(m):
                for t in range(1, 4):
                    rr.dma(m[32 * t : 32 * t + K, :], m[0:K, :])

            LX = _assemble_lhs(nc, rr, mats, px, ones64, "LX")
            RY = _assemble_rhs(nc, rr, mats, py, ones64, "RY")
            replicate(LX)
            replicate(RY)

            identity = parts.tile([128, 128], BF16)
            masks.make_identity(nc, identity[:])

            acc = parts.tile([128, N], BF16)
            nc.vector.memset(acc[:], BIG)
            rowpart = parts.tile([128, NO], F32)
            colpart = parts.tile([128, NO], F32)

            # Each slab r covers rows [128r, 128r+128) of D as two
            # [128, 2048] PSUM units, each filled by four concurrent PE
            # row-group matmuls (tile_position) using the replicated rows.
            def fill_unit(r, h):
                p = psum.tile([128, 2048], F32, name="pp")
                for j in range(4):
                    nc.tensor.matmul(
                        p[:, 512 * j : 512 * (j + 1)],
                        LX[32 * j : 32 * j + K, 128 * r : 128 * (r + 1)],
                        RY[32 * j : 32 * j + K,
                           2048 * h + 512 * j : 2048 * h + 512 * (j + 1)],
                        start=True,
                        stop=True,
                        tile_position=(32 * j, 0),
                    )
                return p

            with (
                tc.tile_pool(name="psum", bufs=2, space="PSUM") as psum,
                tc.tile_pool(name="drain", bufs=3) as drain,
                tc.tile_pool(name="scr", bufs=2) as scr,
            ):
                for r in range(NO):
                    c = drain.tile([128, N], BF16, name="c")
                    for h in range(2):
                        p = fill_unit(r, h)
                        nc.scalar.copy(c[:, 2048 * h : 2048 * (h + 1)], p[:])
                    scratch = scr.tile([128, 2048], BF16, name="scratch")
                    if DEBUG_NO_CUSTOM:
                        nc.vector.tensor_tensor(
                            out=scratch[:], in0=c[:, 0:2048], in1=c[:, 2048:4096],
                            op=mybir.AluOpType.min)
                        t1 = scr.tile([128, 1024], BF16, name="t1")
                        nc.vector.tensor_tensor(
                            out=t1[:], in0=scratch[:, 0:1024], in1=scratch[:, 1024:2048],
                            op=mybir.AluOpType.min)
                        nc.vector.tensor_tensor(
                            out=t1[:, 0:512], in0=t1[:, 0:512], in1=t1[:, 512:1024],
                            op=mybir.AluOpType.min)
                        nc.vector.tensor_reduce(
                            rowpart[:, r : r + 1], t1[:, 0:512],
                            axis=mybir.AxisListType.X, op=mybir.AluOpType.min)
                    else:
                        nc.vector._custom_dve(
                            minmin,
                            out=scratch[:],
                            in0=c[:, 0:2048],
                            in1=c[:, 2048:4096],
                            s0=BIG,
                            accum_out=rowpart[:, r : r + 1],
                        )
                    nc.vector.tensor_tensor(
                        out=acc[:], in0=acc[:], in1=c[:], op=mybir.AluOpType.min
                    )

            # Tail: column minima. acc[p, m] = min over slabs; transpose
            # 128-column chunks (PE keeps bf16 into PSUM) and min-reduce
            # each to get colmin per column block.
            with tc.tile_pool(name="tpsum", bufs=6, space="PSUM") as tpsum:
                for k in range(NO):
                    tp = tpsum.tile([128, 128], BF16, name="tp")
                    nc.tensor.transpose(
                        tp[:], acc[:, 128 * k : 128 * (k + 1)], identity[:]
                    )
                    nc.vector.tensor_reduce(
                        colpart[:, k : k + 1],
                        tp[:],
                        axis=mybir.AxisListType.X,
                        op=mybir.AluOpType.min,
                    )

            osb = parts.tile([128, 2], F32)
            nc.vector.tensor_scalar_max(rowpart[:], rowpart[:], 0.0)
            nc.vector.tensor_scalar_max(colpart[:], colpart[:], 0.0)
            nc.vector.reduce_sum(osb[:, 0:1], rowpart[:], axis=mybir.AxisListType.X)
            nc.vector.reduce_sum(osb[:, 1:2], colpart[:], axis=mybir.AxisListType.X)
            nc.sync.dma_start(out_d[:], osb[:])

    nc.compile()
    return nc


def kernel(ori_pcs: np.ndarray, adv_pcs: np.ndarray) -> np.ndarray:
    if "nc" not in _CACHE:
        _CACHE["nc"] = _build()
    nc = _CACHE["nc"]

    ori = np.ascontiguousarray(np.asarray(ori_pcs, dtype=np.float32))
    adv = np.ascontiguousarray(np.asarray(adv_pcs, dtype=np.float32))
    in_maps = [{"x": ori[b], "y": adv[b]} for b in range(B)]
    res = bass_utils.run_bass_kernel_spmd(nc, in_maps, core_ids=list(range(NCORES)))

    vals = []
    for b in range(B):
        o = res.results[b]["o"].astype(np.float64)
        d1 = o[:, 0].sum() / N
        d2 = o[:, 1].sum() / N
        vals.append(max(d1, d2))
    return np.array(np.mean(vals), dtype=np.float32)


# revision 11
# speedup vs baseline: 1.5879x; 1.2942x over previous
"""Trainium2 Bass kernel for ChamferLoss (B=8, C=3, N=4096), 8 NeuronCores.

Strategy: data-parallel over batch; core b computes batch b fully.
  D[n,m] = ||x_n||^2 + ||y_m||^2 - 2 x_n.y_m   (x = ori, y = adv points)
  d1 = mean_n relu(min_m D),  d2 = mean_m relu(min_n D)
Host combines: mean_b max(d1_b, d2_b).

One-pass design: D is computed once per slab of 128 rows (two [128, 2048]
fp32 PSUM units). The Scalar engine drains each unit to bf16 SBUF (ACT and
DVE are the only engines that can read PSUM, at 1 elem/cycle/partition, so
PSUM traffic is paid exactly once per element). The Vector engine then
  - computes the slab row-min with ONE custom fused DVE op
    (out = min(in0, in1), accum_out = min-reduce of out) — the stock
    TENSOR_TENSOR_REDUCE opcode's firmware table only implements
    mult/add, so a custom table op is registered instead, and
  - folds the slab into a ping-pong column-min accumulator with one
    tensor_tensor min.
The column direction finishes with 32 PE transposes of the accumulator
(bf16 stays bf16 into PSUM) + per-tile min reduces. relu is applied to
the [128, 32] partials at the end (relu commutes with min).

The -2*x.y matmul has contraction K=3; fp32 matmul is 4x slower on PE, so
each fp32 value v is split v = vh + vl (bf16 pair) and the product uses the
3-term expansion  x.y ~= xh.yh + xh.yl + xl.yh  (error ~2^-16 relative).
The squared norms are folded into the same matmul via constant-one rows, so
PSUM holds complete distance values. The [128, 4096] bf16 stationary/moving
images (13 used rows, replicated at partition offsets 32/64/96 so four PE
row-groups run concurrent matmuls via tile_position) are assembled on the
HOST in numpy and DMA'd in directly — this removes ~30us of on-device
prep/assembly ramp.
"""

import os
import sys

sys.path.insert(0, "/opt/trn_rl_repo")

import numpy as np
import ml_dtypes

import concourse.bass as bass  # noqa: F401  (registers engine types)
import concourse.tile as tile
from concourse import bacc, bass_utils, masks, mybir

B, C, N = 8, 3, 4096
NCORES = 8
NO = 32  # n_outer blocks of 128 rows
F32 = mybir.dt.float32
BF16 = mybir.dt.bfloat16
K = 13  # contraction rows: 9 coord product terms + 2 sq rows + 2 one rows
BIG = 3.0e38
BF = ml_dtypes.bfloat16

_CACHE = {}


def _register_minmin_op():
    """Register the fused (min, min-reduce) custom DVE op at runtime.

    out = min(in0, in1); accum_out = min(s0, min over free dim of out).
    Uses the documented custom-DVE extension point (dve_ops.OPS +
    per-NEFF table gen); the sha is self-pinned since this op is defined
    here rather than in the repo's dve_ops registry.
    """
    if "minmin" in _CACHE:
        return _CACHE["minmin"]
    from concourse import dve_ops as dops
    from concourse.dve_spec import Spec, Src0, Src1, C0, minn, lower
    from concourse.dve_uop import DveOpSpec

    name = "CHAMFER_MINMIN_RED"

    def _ref(in0, in1, c0, c1, c2):
        o = np.minimum(in0, in1).astype(np.float32)
        a = np.minimum(c0, o.reshape(o.shape[0], -1).min(axis=-1, keepdims=True))
        return o, a

    spec = Spec(body=minn(Src0, Src1), accum=minn, accum_init=C0, reference=_ref)
    row = dops._CUSTOM_DVE_ROW_BASE + len(dops.OPS)
    shas = {}
    for ver in ("v3", "v4"):
        try:
            s = DveOpSpec(name=name, opcode=row, uops=lower(spec, ver=ver), rd1_en=True)
            shas[ver] = s.sha(ver)
        except Exception:
            pass
    op = dops.DveOp(name, spec, subdim=False, uops_sha=shas)
    dops.OPS.append(op)
    dops.CUSTOM_DVE_SPECS[name] = spec
    dops._SUB_OPCODE_FOR_NAME[name] = row
    _CACHE["minmin"] = op
    return op


def _images(x: np.ndarray, y: np.ndarray):
    """Build the [128, 4096] bf16 lhsT/rhs images on the host.

    Row order (within each 32-partition quadrant replica t at offset 32t):
      lhs rows 0-2 = -2*xh_c, 3-5 = -2*xh_c, 6-8 = -2*xl_c,
          rows 9,10 = ones, rows 11,12 = x2h, x2l
      rhs rows 0-2 =    yh_c, 3-5 =    yl_c, 6-8 =    yh_c,
          rows 9,10 = y2h, y2l, rows 11,12 = ones
    so sum_k lhs[k,i]*rhs[k,j] = -2 x_i.y_j (3-term bf16 expansion)
    + ||y_j||^2 + ||x_i||^2.
    """

    def split(v):
        vh = v.astype(BF)
        vl = (v - vh.astype(np.float32)).astype(BF)
        return vh, vl

    def build(v, lhs):
        vh, vl = split(v)  # [3, N]
        v2 = (v * v).sum(axis=0)  # [N] fp32
        v2h, v2l = split(v2)
        img = np.zeros((128, N), dtype=BF)
        one = np.ones(N, dtype=BF)
        for t in range(4):
            o = 32 * t
            if lhs:
                m2h = (-2.0 * vh.astype(np.float32)).astype(BF)
                m2l = (-2.0 * vl.astype(np.float32)).astype(BF)
                img[o + 0 : o + 3] = m2h
                img[o + 3 : o + 6] = m2h
                img[o + 6 : o + 9] = m2l
                img[o + 9] = one
                img[o + 10] = one
                img[o + 11] = v2h
                img[o + 12] = v2l
            else:
                img[o + 0 : o + 3] = vh
                img[o + 3 : o + 6] = vl
                img[o + 6 : o + 9] = vh
                img[o + 9] = v2h
                img[o + 10] = v2l
                img[o + 11] = one
                img[o + 12] = one
        return img

    return build(x, lhs=True), build(y, lhs=False)


def _build():
    minmin = _register_minmin_op()
    nc = bacc.Bacc("TRN2", target_bir_lowering=False, debug=False)
    lx_d = nc.dram_tensor("lx", [128, N], BF16, kind="ExternalInput").ap()
    ry_d = nc.dram_tensor("ry", [128, N], BF16, kind="ExternalInput").ap()
    out_d = nc.dram_tensor("o", [128, 2], F32, kind="ExternalOutput").ap()

    with tile.TileContext(nc) as tc:
        with (
            tc.tile_pool(name="mats", bufs=1) as mats,
            tc.tile_pool(name="parts", bufs=1) as parts,
        ):
            LX = mats.tile([128, N], BF16, name="LX")
            RY = mats.tile([128, N], BF16, name="RY")
            # Split loads across queues; slab 0 only needs LX columns
            # 0:128 and all of RY, so RY halves go to separate queues.
            nc.sync.dma_start(RY[:, 0:2048], ry_d[:, 0:2048])
            nc.scalar.dma_start(RY[:, 2048:4096], ry_d[:, 2048:4096])
            nc.gpsimd.dma_start(LX[:, 0:1024], lx_d[:, 0:1024])
            nc.sync.dma_start(LX[:, 1024:4096], lx_d[:, 1024:4096])

            identity = parts.tile([128, 128], BF16)
            masks.make_identity(nc, identity[:])

            acc = [parts.tile([128, N], BF16, name=f"acc{i}") for i in range(2)]
            nc.vector.memset(acc[0][:], BIG)
            rowpart = parts.tile([128, NO], F32)
            colpart = parts.tile([128, NO], F32)

            # Each slab r covers rows [128r, 128r+128) of D as two
            # [128, 2048] PSUM units, each filled by four concurrent PE
            # row-group matmuls (tile_position) using the replicated rows.
            def fill_unit(r, h):
                p = psum.tile([128, 2048], F32, name="pp")
                for j in range(4):
                    nc.tensor.matmul(
                        p[:, 512 * j : 512 * (j + 1)],
                        LX[32 * j : 32 * j + K, 128 * r : 128 * (r + 1)],
                        RY[32 * j : 32 * j + K,
                           2048 * h + 512 * j : 2048 * h + 512 * (j + 1)],
                        start=True,
                        stop=True,
                        tile_position=(32 * j, 0),
                    )
                return p

            with (
                tc.tile_pool(name="psum", bufs=2, space="PSUM") as psum,
                tc.tile_pool(name="drain", bufs=3) as drain,
                tc.tile_pool(name="scr", bufs=2) as scr,
            ):
                for r in range(NO):
                    c = drain.tile([128, N], BF16, name="c")
                    for h in range(2):
                        p = fill_unit(r, h)
                        nc.scalar.copy(c[:, 2048 * h : 2048 * (h + 1)], p[:])
                    scratch = scr.tile([128, 2048], BF16, name="scratch")
                    nc.vector._custom_dve(
                        minmin,
                        out=scratch[:],
                        in0=c[:, 0:2048],
                        in1=c[:, 2048:4096],
                        s0=BIG,
                        accum_out=rowpart[:, r : r + 1],
                    )
                    nc.vector.tensor_tensor(
                        out=acc[(r + 1) % 2][:],
                        in0=acc[r % 2][:],
                        in1=c[:],
                        op=mybir.AluOpType.min,
                    )

            # Tail: column minima. acc[p, m] = min over slabs; transpose
            # 128-column chunks (PE keeps bf16 into PSUM) and min-reduce
            # each to get colmin per column block.
            accf = acc[NO % 2]
            with tc.tile_pool(name="tpsum", bufs=6, space="PSUM") as tpsum:
                for k in range(NO):
                    tp = tpsum.tile([128, 128], BF16, name="tp")
                    nc.tensor.transpose(
                        tp[:], accf[:, 128 * k : 128 * (k + 1)], identity[:]
                    )
                    nc.vector.tensor_reduce(
                        colpart[:, k : k + 1],
                        tp[:],
                        axis=mybir.AxisListType.X,
                        op=mybir.AluOpType.min,
                    )

            osb = parts.tile([128, 2], F32)
            nc.vector.tensor_scalar_max(rowpart[:], rowpart[:], 0.0)
            nc.vector.tensor_scalar_max(colpart[:], colpart[:], 0.0)
            nc.vector.reduce_sum(osb[:, 0:1], rowpart[:], axis=mybir.AxisListType.X)
            nc.vector.reduce_sum(osb[:, 1:2], colpart[:], axis=mybir.AxisListType.X)
            nc.sync.dma_start(out_d[:], osb[:])

    nc.compile()
    return nc


def kernel(ori_pcs: np.ndarray, adv_pcs: np.ndarray) -> np.ndarray:
    if "nc" not in _CACHE:
        _CACHE["nc"] = _build()
    nc = _CACHE["nc"]

    ori = np.ascontiguousarray(np.asarray(ori_pcs, dtype=np.float32))
    adv = np.ascontiguousarray(np.asarray(adv_pcs, dtype=np.float32))
    in_maps = []
    for b in range(B):
        lx, ry = _images(ori[b], adv[b])
        in_maps.append({"lx": lx, "ry": ry})
    res = bass_utils.run_bass_kernel_spmd(nc, in_maps, core_ids=list(range(NCORES)))

    vals = []
    for b in range(B):
        o = res.results[b]["o"].astype(np.float64)
        d1 = o[:, 0].sum() / N
        d2 = o[:, 1].sum() / N
        vals.append(max(d1, d2))
    return np.array(np.mean(vals), dtype=np.float32)


# revision 14
# speedup vs baseline: 1.5980x; 1.0064x over previous
"""Trainium2 Bass kernel for ChamferLoss (B=8, C=3, N=4096), 8 NeuronCores.

Strategy: data-parallel over batch; core b computes batch b fully.
  D[n,m] = ||x_n||^2 + ||y_m||^2 - 2 x_n.y_m   (x = ori, y = adv points)
  d1 = mean_n relu(min_m D),  d2 = mean_m relu(min_n D)
Host combines: mean_b max(d1_b, d2_b).

One-pass design: D is computed once per slab of 128 rows (two [128, 2048]
fp32 PSUM units). The Scalar engine drains each unit to bf16 SBUF (ACT and
DVE are the only engines that can read PSUM, at 1 elem/cycle/partition, so
PSUM traffic is paid exactly once per element). The Vector engine then
  - computes the slab row-min with ONE custom fused DVE op
    (out = min(in0, in1), accum_out = min-reduce of out) — the stock
    TENSOR_TENSOR_REDUCE opcode's firmware table only implements
    mult/add, so a custom table op is registered instead, and
  - folds the slab into a ping-pong column-min accumulator with one
    tensor_tensor min.
The column direction finishes with 32 PE transposes of the accumulator
(bf16 stays bf16 into PSUM) + per-tile min reduces. relu is applied to
the [128, 32] partials at the end (relu commutes with min).

The -2*x.y matmul has contraction K=3; fp32 matmul is 4x slower on PE, so
each fp32 value v is split v = vh + vl (bf16 pair) and the product uses the
3-term expansion  x.y ~= xh.yh + xh.yl + xl.yh  (error ~2^-16 relative).
The squared norms are folded into the same matmul via constant-one rows, so
PSUM holds complete distance values. The [128, 4096] bf16 stationary/moving
images (13 used rows, replicated at partition offsets 32/64/96 so four PE
row-groups run concurrent matmuls via tile_position) are assembled on the
HOST in numpy and DMA'd in directly — this removes ~30us of on-device
prep/assembly ramp.
"""

import os
import sys

sys.path.insert(0, "/opt/trn_rl_repo")

import numpy as np
import ml_dtypes

import concourse.bass as bass  # noqa: F401  (registers engine types)
import concourse.tile as tile
from concourse import bacc, bass_utils, masks, mybir

B, C, N = 8, 3, 4096
NCORES = 8
NO = 32  # n_outer blocks of 128 rows
F32 = mybir.dt.float32
BF16 = mybir.dt.bfloat16
K = 13  # contraction rows: 9 coord product terms + 2 sq rows + 2 one rows
BIG = 3.0e38
BF = ml_dtypes.bfloat16

_CACHE = {}


def _register_minmin_op():
    """Register the fused (min, min-reduce) custom DVE op at runtime.

    out = min(in0, in1); accum_out = min(s0, min over free dim of out).
    Uses the documented custom-DVE extension point (dve_ops.OPS +
    per-NEFF table gen); the sha is self-pinned since this op is defined
    here rather than in the repo's dve_ops registry.
    """
    if "minmin" in _CACHE:
        return _CACHE["minmin"]
    from concourse import dve_ops as dops
    from concourse.dve_spec import Spec, Src0, Src1, C0, minn, lower
    from concourse.dve_uop import DveOpSpec

    name = "CHAMFER_MINMIN_RED"

    def _ref(in0, in1, c0, c1, c2):
        o = np.minimum(in0, in1).astype(np.float32)
        a = np.minimum(c0, o.reshape(o.shape[0], -1).min(axis=-1, keepdims=True))
        return o, a

    spec = Spec(body=minn(Src0, Src1), accum=minn, accum_init=C0, reference=_ref)
    row = dops._CUSTOM_DVE_ROW_BASE + len(dops.OPS)
    shas = {}
    for ver in ("v3", "v4"):
        try:
            s = DveOpSpec(name=name, opcode=row, uops=lower(spec, ver=ver), rd1_en=True)
            shas[ver] = s.sha(ver)
        except Exception:
            pass
    op = dops.DveOp(name, spec, subdim=False, uops_sha=shas)
    dops.OPS.append(op)
    dops.CUSTOM_DVE_SPECS[name] = spec
    dops._SUB_OPCODE_FOR_NAME[name] = row
    _CACHE["minmin"] = op
    return op


def _images(x: np.ndarray, y: np.ndarray):
    """Build the [128, 4096] bf16 lhsT/rhs images on the host.

    Row order (within each 32-partition quadrant replica t at offset 32t):
      lhs rows 0-2 = -2*xh_c, 3-5 = -2*xh_c, 6-8 = -2*xl_c,
          rows 9,10 = ones, rows 11,12 = x2h, x2l
      rhs rows 0-2 =    yh_c, 3-5 =    yl_c, 6-8 =    yh_c,
          rows 9,10 = y2h, y2l, rows 11,12 = ones
    so sum_k lhs[k,i]*rhs[k,j] = -2 x_i.y_j (3-term bf16 expansion)
    + ||y_j||^2 + ||x_i||^2.
    """

    def split(v):
        vh = v.astype(BF)
        vl = (v - vh.astype(np.float32)).astype(BF)
        return vh, vl

    def build(v, lhs):
        vh, vl = split(v)  # [3, N]
        v2 = (v * v).sum(axis=0)  # [N] fp32
        v2h, v2l = split(v2)
        img = np.zeros((128, N), dtype=BF)
        one = np.ones(N, dtype=BF)
        for t in range(4):
            o = 32 * t
            if lhs:
                m2h = (-2.0 * vh.astype(np.float32)).astype(BF)
                m2l = (-2.0 * vl.astype(np.float32)).astype(BF)
                img[o + 0 : o + 3] = m2h
                img[o + 3 : o + 6] = m2h
                img[o + 6 : o + 9] = m2l
                img[o + 9] = one
                img[o + 10] = one
                img[o + 11] = v2h
                img[o + 12] = v2l
            else:
                img[o + 0 : o + 3] = vh
                img[o + 3 : o + 6] = vl
                img[o + 6 : o + 9] = vh
                img[o + 9] = v2h
                img[o + 10] = v2l
                img[o + 11] = one
                img[o + 12] = one
        return img

    return build(x, lhs=True), build(y, lhs=False)


def _build():
    minmin = _register_minmin_op()
    nc = bacc.Bacc("TRN2", target_bir_lowering=False, debug=False)
    lx_d = nc.dram_tensor("lx", [128, N], BF16, kind="ExternalInput").ap()
    ry_d = nc.dram_tensor("ry", [128, N], BF16, kind="ExternalInput").ap()
    out_d = nc.dram_tensor("o", [128, 2], F32, kind="ExternalOutput").ap()

    with tile.TileContext(nc) as tc:
        with (
            tc.tile_pool(name="mats", bufs=1) as mats,
            tc.tile_pool(name="parts", bufs=1) as parts,
        ):
            LX = mats.tile([128, N], BF16, name="LX")
            RY = mats.tile([128, N], BF16, name="RY")
            # Split loads across queues; slab 0 only needs LX columns
            # 0:128 and all of RY, so RY halves go to separate queues.
            nc.sync.dma_start(RY[:, 0:2048], ry_d[:, 0:2048])
            nc.scalar.dma_start(RY[:, 2048:4096], ry_d[:, 2048:4096])
            nc.gpsimd.dma_start(LX[:, 0:1024], lx_d[:, 0:1024])
            nc.gpsimd.dma_start(LX[:, 1024:4096], lx_d[:, 1024:4096])

            identity = parts.tile([128, 128], BF16)
            masks.make_identity(nc, identity[:])

            acc = [parts.tile([128, N], BF16, name=f"acc{i}") for i in range(2)]
            nc.vector.memset(acc[0][:], BIG)
            rowpart = parts.tile([128, NO], F32)
            colpart = parts.tile([128, NO], F32)

            # Each slab r covers rows [128r, 128r+128) of D as two
            # [128, 2048] PSUM units, each filled by four concurrent PE
            # row-group matmuls (tile_position) using the replicated rows.
            def fill_unit(r, h):
                p = psum.tile([128, 2048], F32, name="pp")
                for j in range(4):
                    nc.tensor.matmul(
                        p[:, 512 * j : 512 * (j + 1)],
                        LX[32 * j : 32 * j + K, 128 * r : 128 * (r + 1)],
                        RY[32 * j : 32 * j + K,
                           2048 * h + 512 * j : 2048 * h + 512 * (j + 1)],
                        start=True,
                        stop=True,
                        tile_position=(32 * j, 0),
                    )
                return p

            with (
                tc.tile_pool(name="psum", bufs=2, space="PSUM") as psum,
                tc.tile_pool(name="drain", bufs=4) as drain,
                tc.tile_pool(name="scr", bufs=2) as scr,
            ):
                for r in range(NO):
                    c = drain.tile([128, N], BF16, name="c")
                    for h in range(2):
                        p = fill_unit(r, h)
                        nc.scalar.copy(c[:, 2048 * h : 2048 * (h + 1)], p[:])
                    scratch = scr.tile([128, 2048], BF16, name="scratch")
                    nc.vector._custom_dve(
                        minmin,
                        out=scratch[:],
                        in0=c[:, 0:2048],
                        in1=c[:, 2048:4096],
                        s0=BIG,
                        accum_out=rowpart[:, r : r + 1],
                    )
                    nc.vector.tensor_tensor(
                        out=acc[(r + 1) % 2][:],
                        in0=acc[r % 2][:],
                        in1=c[:],
                        op=mybir.AluOpType.min,
                    )

            # Tail: column minima. acc[p, m] = min over slabs; transpose
            # 128-column chunks (PE keeps bf16 into PSUM) and min-reduce
            # each to get colmin per column block.
            accf = acc[NO % 2]
            with tc.tile_pool(name="tpsum", bufs=8, space="PSUM") as tpsum:
                for k in range(NO):
                    tp = tpsum.tile([128, 128], BF16, name="tp")
                    nc.tensor.transpose(
                        tp[:], accf[:, 128 * k : 128 * (k + 1)], identity[:]
                    )
                    nc.vector.tensor_reduce(
                        colpart[:, k : k + 1],
                        tp[:],
                        axis=mybir.AxisListType.X,
                        op=mybir.AluOpType.min,
                    )

            osb = parts.tile([128, 2], F32)
            nc.vector.tensor_scalar_max(rowpart[:], rowpart[:], 0.0)
            nc.vector.tensor_scalar_max(colpart[:], colpart[:], 0.0)
            nc.vector.reduce_sum(osb[:, 0:1], rowpart[:], axis=mybir.AxisListType.X)
            nc.vector.reduce_sum(osb[:, 1:2], colpart[:], axis=mybir.AxisListType.X)
            nc.sync.dma_start(out_d[:], osb[:])

    nc.compile()
    return nc


def kernel(ori_pcs: np.ndarray, adv_pcs: np.ndarray) -> np.ndarray:
    if "nc" not in _CACHE:
        _CACHE["nc"] = _build()
    nc = _CACHE["nc"]

    ori = np.ascontiguousarray(np.asarray(ori_pcs, dtype=np.float32))
    adv = np.ascontiguousarray(np.asarray(adv_pcs, dtype=np.float32))
    in_maps = []
    for b in range(B):
        lx, ry = _images(ori[b], adv[b])
        in_maps.append({"lx": lx, "ry": ry})
    res = bass_utils.run_bass_kernel_spmd(nc, in_maps, core_ids=list(range(NCORES)))

    vals = []
    for b in range(B):
        o = res.results[b]["o"].astype(np.float64)
        d1 = o[:, 0].sum() / N
        d2 = o[:, 1].sum() / N
        vals.append(max(d1, d2))
    return np.array(np.mean(vals), dtype=np.float32)
